# revision 8
# baseline (speedup 1.0000x reference)
"""Trainium2 Bass kernel for nn_EternalRecursion (GRUCell self-recursion, B=512, D=500).

Strategy
--------
Data-parallel over 8 NeuronCores: 64 batch rows per core, GRU weights replicated.

Math restructuring (host-side):
  - After step 1 the reference feeds h_new as BOTH x and h of the GRU cell, so
    steps >= 2 use combined weights W_rz = (W_ih+W_hh)[0:1000] for the r/z gates,
    while the n-gate keeps W_ih_n / W_hh_n separate (r multiplies only the h-side).
  - Step 1 (x=state, h=0) uses W_ih with a zero block for the h-side n columns:
    same device code path, different weights.
  - Biases fold into the matmuls via an extra contraction row of ones.
  - h_new = n + z*(h-n), so the transposed next-step stationary is computed on
    the PE as T(h_new) = T(n) + T(z*(h-n)) by two accumulating fp16 matmuls,
    keeping the post-matmul serial chain to sigmoid -> mul -> transpose -> copy.

Precision (validated vs the f32 reference: ~1.2e-3 max rel err):
  - r/z gate matmuls run in fp8e4m3 with DoubleRow perf mode (2 K-tiles per
    matmul at 2 rows/cycle). The sigmoid's <=1/4 slope absorbs fp8 noise.
  - n-gate matmuls and the h_new transposes run in fp16.
  - Gate math, carried h state, and the break-check sums stay fp32.

Device layout (per core, per step):
  - h packed [128, 250]: partition 64*H+b holds h[b, 250*H + c].
  - rz PSUM [128, 500] = [r | z], from 4 DoubleRow matmuls (pairs of the 8
    doubled-K groups); n PSUM split per chunk c: [gin_c | ghn_c] [128, 250]
    in a full bank each, from 8 fp16 matmuls per chunk.
  - Software pipeline: matmuls needing transpose-A (h cols 0:125 -> K-groups
    D0/D1) are emitted first each step; chunk-1's transposes + copies of step
    k run during step k+1's prefix, so the serial gate chain of one chunk
    always overlaps the other chunk's / next step's matmul stream.
"""

import os
import sys
import types
import numpy as np
import ml_dtypes

D = 500
B = 512
NCORES = 8
BS = B // NCORES          # 64 batch rows per core
HALF = 250                # free columns of the packed layout
F8NP = ml_dtypes.float8_e4m3fn

# K permutation: hT row-groups are [0:125 | 250:375 | 125:250 | 375:500]
PERM = np.concatenate([
    np.arange(0, 125), np.arange(250, 375),
    np.arange(125, 250), np.arange(375, 500),
])
TBLK = (0, 1, 0, 1, 2, 3, 2, 3)    # n-gate group u -> D-block t
GHALF = (0, 0, 1, 1, 0, 0, 1, 1)   # n-gate group u -> gate half g
PAIR_G = (0, 1, 0, 1)              # rz DoubleRow pair p -> gate half g
PAIR_T = ((0, 1), (0, 1), (2, 3), (2, 3))  # pair p -> (D-block of i=0, i=1)


def _install_hook_module():
    """Provide antenv.axon_hooks (missing from the RO image) so NTFF tracing
    through bass_utils can work when requested. Harmless if anything fails."""
    if "antenv.axon_hooks" in sys.modules:
        return
    mod = types.ModuleType("antenv.axon_hooks")
    holder = [None]
    mod.set_axon_ntff_profile_hook = lambda h: holder.__setitem__(0, h)
    mod.get_axon_ntff_profile_hook = lambda: holder[0]
    sys.modules["antenv.axon_hooks"] = mod
    try:
        from trn_agent_boot.trn_boot import _ntff_profile_via_ctypes
        hook = _ntff_profile_via_ctypes("/opt/axon/libaxon_pjrt.so")
        mod.set_axon_ntff_profile_hook(hook)
    except Exception:
        pass


_install_hook_module()

import concourse.bass as bass  # noqa: E402
import concourse.mybir as mybir  # noqa: E402
import concourse.tile as tile  # noqa: E402
from concourse import bass_utils  # noqa: E402
from concourse.masks import make_identity  # noqa: E402
import bass_rust  # noqa: E402

F32 = mybir.dt.float32
F16 = mybir.dt.float16
F8 = mybir.dt.float8e4
AF = mybir.ActivationFunctionType
ALU = mybir.AluOpType
DR = mybir.MatmulPerfMode.DoubleRow


def _split_overwide_waits(nc, maxw=1):
    """walrus here rejects >1 sync wait per instruction; spread extras over
    preceding NoOp carriers. Most multi-wait instructions get same-engine
    carriers (order-preserving); the kernel-end drain (many loose-end waits)
    gets carriers round-robined across all engines so they resolve in
    parallel before the final barrier instead of serially on one engine."""
    n_new = 0
    all_engines = (mybir.EngineType.SP, mybir.EngineType.Activation,
                   mybir.EngineType.PE, mybir.EngineType.DVE,
                   mybir.EngineType.Pool)
    for fn in nc.m.functions:
        for bb in fn.blocks:
            out = []
            for inst in bb.instructions:
                si = inst.sync_info
                if si is not None and si.on_wait and len(si.on_wait) > maxw:
                    waits = list(si.on_wait)
                    chunks = [waits[i:i + maxw] for i in range(0, len(waits), maxw)]
                    spread = len(chunks) > 4  # only the big end-of-kernel drain
                    for j, ch in enumerate(chunks[:-1]):
                        eng = all_engines[j % len(all_engines)] if spread \
                            else inst.engine
                        nd = mybir.InstNoOp(
                            name=f"I-swx{n_new}", engine=eng,
                            bass_nofuse=True,
                            sync_info=bass_rust.SyncInfo(on_wait=ch, on_update=[]))
                        n_new += 1
                        nc.register_instruction(nd, overwrite=True)
                        out.append(nd)
                    inst.sync_info = bass_rust.SyncInfo(
                        on_wait=chunks[-1], on_update=list(si.on_update or []))
                out.append(inst)
            bb.instructions = out
    return n_new


def _build(L):
    """Build the Bass module for L GRU steps. Returns nc."""
    assert L >= 1
    nc = bass.Bass("TRN2", target_bir_lowering=False, debug=False)

    statet16_d = nc.dram_tensor("statet16", [126, 1024], F16, kind="ExternalInput").ap()
    statet8_d = nc.dram_tensor("statet8", [126, 1024], F8, kind="ExternalInput").ap()
    wn_d = nc.dram_tensor("wn", [2, 126, 4000], F16, kind="ExternalInput").ap()
    wrz_d = nc.dram_tensor("wrz", [2, 126, 4000], F8, kind="ExternalInput").ap()
    hout_d = nc.dram_tensor("hout", [128, HALF], F32, kind="ExternalOutput").ap()
    sums_d = nc.dram_tensor("sums", [128, 4 * L], F32, kind="ExternalOutput").ap()

    with tile.TileContext(nc) as tc:
        import contextlib
        with contextlib.ExitStack() as ctx:
            consts = ctx.enter_context(tc.tile_pool(name="consts", bufs=1))
            wpool = ctx.enter_context(tc.tile_pool(name="weights", bufs=1))
            hpool = ctx.enter_context(tc.tile_pool(name="hstate", bufs=1))
            work = ctx.enter_context(tc.tile_pool(name="work", bufs=2))
            gpsum = ctx.enter_context(tc.tile_pool(name="gpsum", bufs=2, space="PSUM"))
            tpsum = ctx.enter_context(tc.tile_pool(name="tpsum", bufs=1, space="PSUM"))

            identity = consts.tile([128, 128], F32, tag="identity", name="identity")
            make_identity(nc, identity[:])
            ident16 = consts.tile([128, 128], F16, tag="ident16", name="ident16")
            nc.vector.tensor_copy(ident16[:], identity[:])

            statet16 = wpool.tile([126, 1024], F16, tag="statet16", name="statet16")
            statet8 = wpool.tile([126, 1024], F8, tag="statet8", name="statet8")
            nc.gpsimd.dma_start(statet16[:], statet16_d)
            nc.gpsimd.dma_start(statet8[:], statet8_d)
            wn_t = [wpool.tile([126, 4000], F16, tag=f"wn{j}", name=f"wn{j}")
                    for j in range(2)]
            wrz_t = [wpool.tile([126, 4000], F8, tag=f"wrz{j}", name=f"wrz{j}")
                     for j in range(2)]
            nc.sync.dma_start(wrz_t[0][:], wrz_d[0])
            nc.gpsimd.dma_start(wn_t[0][:], wn_d[0])
            nc.sync.dma_start(wrz_t[1][:], wrz_d[1])
            nc.gpsimd.dma_start(wn_t[1][:], wn_d[1])

            hT16 = [hpool.tile([126, 1024], F16, tag=f"ht16{i}", name=f"ht16{i}")
                    for i in range(2)]
            hT8 = [hpool.tile([126, 1024], F8, tag=f"ht8{i}", name=f"ht8{i}")
                   for i in range(2)]
            h32 = [hpool.tile([128, HALF], F32, tag=f"h32{i}", name=f"h32{i}")
                   for i in range(2)]
            # zero the data rows; DMA row 125 (the bias ones live there) from
            # the state images (DVE ops can't start at partition 125, DMA can)
            for i in range(2):
                nc.vector.memzero(hT16[i][0:125, :])
                nc.vector.memzero(hT8[i][0:125, :])
                nc.gpsimd.dma_start(hT16[i][125:126, :], statet16_d[125:126, :])
                nc.gpsimd.dma_start(hT8[i][125:126, :], statet8_d[125:126, :])
            nc.vector.memzero(h32[1][:])

            sums = consts.tile([128, 4 * L], F32, tag="sums", name="sums")

            prev_n16 = None
            prev_zt16 = None
            for k in range(1, L + 1):
                first = k == 1
                rdbuf = k % 2
                dstbuf = (k + 1) % 2
                lhs16 = statet16 if first else hT16[rdbuf]
                lhs8 = statet8 if first else hT8[rdbuf]
                w16 = wn_t[0 if first else 1]
                w8 = wrz_t[0 if first else 1]

                rzp = gpsum.tile([128, 512], F32, tag="rzp", name="rzp")
                gnp = [gpsum.tile([128, 512], F32, tag=f"gnp{c}", name=f"gnp{c}")
                       for c in range(2)]

                def rz_mm(p, start, stop):
                    Kp = 126 if p < 2 else 125
                    lt = lhs8[0:Kp, 256 * p:256 * p + 256].rearrange(
                        "p (i c) -> p i c", c=128)
                    rt = w8[0:Kp, 1000 * p:1000 * p + 1000].rearrange(
                        "p (i n) -> p i n", n=500)
                    nc.tensor.matmul(rzp[:, 0:500], lt, rt, start=start,
                                     stop=stop, perf_mode=DR,
                                     skip_group_check=True)

                def n_mm(u, c, start, stop):
                    Ku = 126 if u in (1, 3) else 125
                    lt = lhs16[0:Ku, 128 * u:128 * u + 128]
                    rt = w16[0:Ku, 500 * u + 250 * c:500 * u + 250 * c + 250]
                    nc.tensor.matmul(gnp[c][:, 0:250], lt, rt, start=start,
                                     stop=stop, skip_group_check=True)

                def t_mm(dst, src_ap, start, stop):
                    nc.tensor.matmul(dst, src_ap, ident16[:], start=start,
                                     stop=stop, skip_group_check=True)

                def copies(tp, half, dst16, dst8):
                    """Copy the [125,128] transpose PSUM into the hT16 slot
                    halves (half 0 -> groups 0-3 cols 0:512, half 1 -> groups
                    4-7 cols 512:1024), then refresh the fp8 stationary: its
                    pair-major layout matches hT16's group-major layout
                    column-for-column, so one contiguous cast covers it."""
                    tv = tp[:].rearrange("p (u c) -> p u c", c=64)
                    o = 512 * half
                    d0_16 = dst16[0:125, o:o + 256].rearrange(
                        "p (u c) -> p u c", c=128)[:, :, 0:64]
                    d1_16 = dst16[0:125, o + 256:o + 512].rearrange(
                        "p (u c) -> p u c", c=128)[:, :, 64:128]
                    nc.scalar.copy(d0_16, tv)
                    nc.vector.tensor_copy(d1_16, tv)
                    nc.gpsimd.tensor_copy(dst8[0:125, o:o + 512],
                                          dst16[0:125, o:o + 512])

                # ---- prefix: matmuls that only need transpose-A of h_{k-1},
                # plus chunk-1 transposes+copies of step k-1 ----
                if prev_n16 is not None:
                    tpB = tpsum.tile([125, 128], F32, tag="tpB", name="tpB")
                    t_mm(tpB[:], prev_n16[:, 125:250], True, False)
                rz_mm(0, True, False)
                rz_mm(1, False, False)
                if prev_n16 is not None:
                    t_mm(tpB[:], prev_zt16[:, 125:250], False, True)
                    copies(tpB, 1, hT16[rdbuf], hT8[rdbuf])
                for u in (0, 1, 2, 3):
                    n_mm(u, 0, u == 0, False)
                for u in (0, 1, 2, 3):
                    n_mm(u, 1, u == 0, False)

                # ---- suffix: matmuls needing transpose-B ----
                rz_mm(2, False, False)
                rz_mm(3, False, True)
                for u in (4, 5, 6, 7):
                    n_mm(u, 0, False, u == 7)
                for u in (4, 5, 6, 7):
                    n_mm(u, 1, False, u == 7)

                # ---- gate chains ----
                rz = work.tile([128, 2 * HALF], F32, tag="rz", name="rz")
                nc.scalar.activation(rz[:], rzp[:, 0:500], AF.Sigmoid)
                r = rz[:, 0:250]
                z = rz[:, 250:500]
                rhn = work.tile([128, HALF], F32, tag="rhn", name="rhn")
                targ = work.tile([128, HALF], F32, tag="targ", name="targ")
                hmn = work.tile([128, HALF], F32, tag="hmn", name="hmn")
                n16 = work.tile([128, HALF], F16, tag="n16", name="n16")
                zt16 = work.tile([128, HALF], F16, tag="zt16", name="zt16")
                for c in (0, 1):
                    cs = slice(125 * c, 125 * (c + 1))
                    acc_n = sums[:, 4 * (k - 1) + c:4 * (k - 1) + c + 1]
                    acc_z = sums[:, 4 * (k - 1) + 2 + c:4 * (k - 1) + 3 + c]
                    nc.vector.tensor_mul(rhn[:, cs], r[:, cs], gnp[c][:, 125:250])
                    nc.vector.tensor_add(targ[:, cs], rhn[:, cs], gnp[c][:, 0:125])
                    nc.scalar.activation(n16[:, cs], targ[:, cs], AF.Tanh,
                                         accum_out=acc_n)
                    nc.gpsimd.tensor_sub(hmn[:, cs], h32[rdbuf][:, cs], n16[:, cs])
                    nc.vector.scalar_tensor_tensor(
                        zt16[:, cs], z[:, cs], 1.0, hmn[:, cs],
                        op0=ALU.mult, op1=ALU.mult, accum_out=acc_z)
                    nc.gpsimd.tensor_add(h32[dstbuf][:, cs], n16[:, cs],
                                         zt16[:, cs])

                if k < L:
                    tpA = tpsum.tile([125, 128], F32, tag="tpA", name="tpA")
                    t_mm(tpA[:], n16[:, 0:125], True, False)
                    t_mm(tpA[:], zt16[:, 0:125], False, True)
                    copies(tpA, 0, hT16[dstbuf], hT8[dstbuf])
                    prev_n16, prev_zt16 = n16, zt16
                else:
                    prev_n16 = prev_zt16 = None

            nc.gpsimd.dma_start(hout_d, h32[(L + 1) % 2][:])
            nc.gpsimd.dma_start(sums_d, sums[:])

    _split_overwide_waits(nc)
    return nc


_NC_CACHE = {}


def _get_nc(L):
    if L not in _NC_CACHE:
        _NC_CACHE[L] = _build(L)
    return _NC_CACHE[L]


def _prep_weights(W_ih, W_hh, b_ih, b_hh):
    """Build the DRAM weight images: wrz [2, 126, 4000] fp8 (DoubleRow pair
    layout) and wn [2, 126, 4000] fp16 (chunk-interleaved n-gate layout).
    Index 0 = step-1 (x=state, h=0) weights, 1 = steady-state weights."""
    W_ih = np.asarray(W_ih, np.float32)
    W_hh = np.asarray(W_hh, np.float32)
    b_ih = np.asarray(b_ih, np.float32)
    b_hh = np.asarray(b_hh, np.float32)

    def rz_img(Wrz, brz):
        img = np.zeros((126, 4000), np.float32)
        for p in range(4):
            g = PAIR_G[p]
            rows = np.concatenate([np.arange(250 * g, 250 * g + 250),
                                   np.arange(500 + 250 * g, 500 + 250 * g + 250)])
            for i, t in enumerate(PAIR_T[p]):
                cols = PERM[125 * t:125 * (t + 1)]
                img[0:125, 1000 * p + 500 * i:1000 * p + 500 * i + 500] = \
                    Wrz[np.ix_(rows, cols)].T
            if p < 2:
                img[125, 1000 * p + 500:1000 * p + 1000] = brz[rows]
        return img.astype(F8NP)

    def n_img(Win, Whn, bin_, bhn):
        img = np.zeros((126, 4000), np.float32)
        for u in range(8):
            t, g = TBLK[u], GHALF[u]
            cols = PERM[125 * t:125 * (t + 1)]
            base = 500 * u
            for c in range(2):
                ch = np.arange(250 * g + 125 * c, 250 * g + 125 * c + 125)
                img[0:125, base + 250 * c:base + 250 * c + 125] = \
                    Win[np.ix_(ch, cols)].T
                img[0:125, base + 250 * c + 125:base + 250 * c + 250] = \
                    Whn[np.ix_(ch, cols)].T
                if u in (1, 3):
                    img[125, base + 250 * c:base + 250 * c + 125] = bin_[ch]
                    img[125, base + 250 * c + 125:base + 250 * c + 250] = bhn[ch]
        return img.astype(np.float16)

    Win = W_ih[1000:1500]
    Whn = W_hh[1000:1500]
    zeros_w = np.zeros_like(Whn)
    zeros_b = np.zeros(500, np.float32)
    wrz = np.stack([rz_img(W_ih[:1000], b_ih[:1000]),
                    rz_img(W_ih[:1000] + W_hh[:1000], b_ih[:1000] + b_hh[:1000])])
    wn = np.stack([n_img(Win, zeros_w, b_ih[1000:1500], zeros_b),
                   n_img(Win, Whn, b_ih[1000:1500], b_hh[1000:1500])])
    return np.ascontiguousarray(wrz), np.ascontiguousarray(wn)


def _prep_state(state):
    """Per-core stationary state^T images: fp16 [126, 1024] (group-major) and
    fp8 [126, 1024] (DoubleRow pair-major)."""
    state = np.asarray(state, np.float32)
    outs = []
    for cidx in range(NCORES):
        shard = state[BS * cidx:BS * (cidx + 1)]      # [64, 500]
        st = shard[:, PERM].T                          # [500, 64]
        s16 = np.zeros((126, 1024), np.float32)
        for u in range(8):
            t, g = TBLK[u], GHALF[u]
            s16[0:125, 128 * u + 64 * g:128 * u + 64 * g + 64] = \
                st[125 * t:125 * (t + 1)]
        s16[125, 128 * 1:128 * 1 + 64] = 1.0
        s16[125, 128 * 3 + 64:128 * 3 + 128] = 1.0
        s8 = np.zeros((126, 1024), np.float32)
        for p in range(4):
            g = PAIR_G[p]
            for i, t in enumerate(PAIR_T[p]):
                off = 256 * p + 128 * i + 64 * g
                s8[0:125, off:off + 64] = st[125 * t:125 * (t + 1)]
            if p < 2:
                s8[125, 256 * p + 128 + 64 * g:256 * p + 128 + 64 * g + 64] = 1.0
        outs.append((s16.astype(np.float16), s8.astype(F8NP)))
    return outs


def _run(L, stateTs, wrz, wn, trace=False):
    nc = _get_nc(L)
    in_maps = [{"statet16": np.ascontiguousarray(stateTs[c][0]),
                "statet8": np.ascontiguousarray(stateTs[c][1]),
                "wrz": wrz, "wn": wn} for c in range(NCORES)]
    res = bass_utils.run_bass_kernel_spmd(
        nc, in_maps, core_ids=list(range(NCORES)), trace=trace)
    shards = []
    sums = np.zeros((128, 4 * L), np.float64)
    for c in range(NCORES):
        hout = res.results[c]["hout"]
        shards.append(np.concatenate([hout[0:64], hout[64:128]], axis=1))
        sums += res.results[c]["sums"].astype(np.float64)
    h = np.concatenate(shards, axis=0)                # [512, 500]
    means = sums.reshape(128, L, 4).sum(axis=(0, 2)) / (B * D)     # [L]
    return h, means, res


def kernel(state, W_ih, W_hh, b_ih, b_hh, break_condition, recursion_limit):
    state = np.asarray(state, np.float32)
    L = int(np.asarray(recursion_limit))
    if L <= 0:
        return state.copy()
    bc = float(np.asarray(break_condition))

    wrz, wn = _prep_weights(W_ih, W_hh, b_ih, b_hh)
    stateTs = _prep_state(state)

    h, means, _ = _run(L, stateTs, wrz, wn)
    fired = np.nonzero(means > bc)[0]
    if fired.size and fired[0] + 1 < L:
        # break fired at step k* = fired[0]+1: output latches h_{k*}
        h, _, _ = _run(int(fired[0]) + 1, stateTs, wrz, wn)
    return h.astype(np.float32)


# revision 11
# speedup vs baseline: 1.5575x; 1.5575x over previous
"""Trainium2 Bass kernel for nn_EternalRecursion (GRUCell self-recursion, B=512, D=500).

Strategy
--------
Data-parallel over 8 NeuronCores: 64 batch rows per core, GRU weights replicated.

Math restructuring (host-side):
  - After step 1 the reference feeds h_new as BOTH x and h of the GRU cell, so
    steps >= 2 use combined weights W_rz = (W_ih+W_hh)[0:1000] for the r/z gates,
    while the n-gate keeps W_ih_n / W_hh_n separate (r multiplies only the h-side).
  - Step 1 (x=state, h=0) uses W_ih with a zero block for the h-side n columns:
    same device code path, different weights.
  - Biases fold into the matmuls via an extra contraction row of ones.
  - h_new = n + z*(h-n), so the transposed next-step stationary is computed on
    the PE as T(h_new) = T(n) + T(z*(h-n)) by two accumulating fp16 matmuls,
    keeping the post-matmul serial chain to sigmoid -> mul -> transpose -> copy.

Precision (validated vs the f32 reference: ~1.2e-3 max rel err):
  - r/z gate matmuls run in fp8e4m3 with DoubleRow perf mode (2 K-tiles per
    matmul at 2 rows/cycle). The sigmoid's <=1/4 slope absorbs fp8 noise.
  - n-gate matmuls and the h_new transposes run in fp16.
  - Gate math, carried h state, and the break-check sums stay fp32.

Device layout (per core, per step):
  - h packed [128, 250]: partition 64*H+b holds h[b, 250*H + c].
  - rz PSUM [128, 500] = [r | z], from 4 DoubleRow matmuls (pairs of the 8
    doubled-K groups); n PSUM split per chunk c: [gin_c | ghn_c] [128, 250]
    in a full bank each, from 8 fp16 matmuls per chunk.
  - Software pipeline: matmuls needing transpose-A (h cols 0:125 -> K-groups
    D0/D1) are emitted first each step; chunk-1's transposes + copies of step
    k run during step k+1's prefix, so the serial gate chain of one chunk
    always overlaps the other chunk's / next step's matmul stream.
"""

import os
import sys
import types
import numpy as np
import ml_dtypes

D = 500
B = 512
NCORES = 8
BS = B // NCORES          # 64 batch rows per core
HALF = 250                # free columns of the packed layout
F8NP = ml_dtypes.float8_e4m3fn

# K permutation: hT row-groups are [0:125 | 250:375 | 125:250 | 375:500]
PERM = np.concatenate([
    np.arange(0, 125), np.arange(250, 375),
    np.arange(125, 250), np.arange(375, 500),
])
TBLK = (0, 1, 0, 1, 2, 3, 2, 3)    # n-gate group u -> D-block t
GHALF = (0, 0, 1, 1, 0, 0, 1, 1)   # n-gate group u -> gate half g
PAIR_G = (0, 1, 0, 1)              # rz DoubleRow pair p -> gate half g
PAIR_T = ((0, 1), (0, 1), (2, 3), (2, 3))  # pair p -> (D-block of i=0, i=1)


def _install_hook_module():
    """Provide antenv.axon_hooks (missing from the RO image) so NTFF tracing
    through bass_utils can work when requested. Harmless if anything fails."""
    if "antenv.axon_hooks" in sys.modules:
        return
    mod = types.ModuleType("antenv.axon_hooks")
    holder = [None]
    mod.set_axon_ntff_profile_hook = lambda h: holder.__setitem__(0, h)
    mod.get_axon_ntff_profile_hook = lambda: holder[0]
    sys.modules["antenv.axon_hooks"] = mod
    try:
        from trn_agent_boot.trn_boot import _ntff_profile_via_ctypes
        hook = _ntff_profile_via_ctypes("/opt/axon/libaxon_pjrt.so")
        mod.set_axon_ntff_profile_hook(hook)
    except Exception:
        pass


_install_hook_module()

import concourse.bass as bass  # noqa: E402
import concourse.mybir as mybir  # noqa: E402
import concourse.tile as tile  # noqa: E402
from concourse import bass_utils  # noqa: E402
from concourse.masks import make_identity  # noqa: E402
import bass_rust  # noqa: E402

F32 = mybir.dt.float32
F16 = mybir.dt.float16
F8 = mybir.dt.float8e4
AF = mybir.ActivationFunctionType
ALU = mybir.AluOpType
DR = mybir.MatmulPerfMode.DoubleRow


def _split_overwide_waits(nc, maxw=1):
    """walrus here rejects >1 sync wait per instruction; spread extras over
    preceding NoOp carriers. Most multi-wait instructions get same-engine
    carriers (order-preserving); the kernel-end drain (many loose-end waits)
    gets carriers round-robined across all engines so they resolve in
    parallel before the final barrier instead of serially on one engine."""
    n_new = 0
    all_engines = (mybir.EngineType.SP, mybir.EngineType.Activation,
                   mybir.EngineType.PE, mybir.EngineType.DVE,
                   mybir.EngineType.Pool)
    for fn in nc.m.functions:
        for bb in fn.blocks:
            out = []
            for inst in bb.instructions:
                si = inst.sync_info
                if si is not None and si.on_wait and len(si.on_wait) > maxw:
                    waits = list(si.on_wait)
                    chunks = [waits[i:i + maxw] for i in range(0, len(waits), maxw)]
                    spread = len(chunks) > 4  # only the big end-of-kernel drain
                    for j, ch in enumerate(chunks[:-1]):
                        eng = all_engines[j % len(all_engines)] if spread \
                            else inst.engine
                        nd = mybir.InstNoOp(
                            name=f"I-swx{n_new}", engine=eng,
                            bass_nofuse=True,
                            sync_info=bass_rust.SyncInfo(on_wait=ch, on_update=[]))
                        n_new += 1
                        nc.register_instruction(nd, overwrite=True)
                        out.append(nd)
                    inst.sync_info = bass_rust.SyncInfo(
                        on_wait=chunks[-1], on_update=list(si.on_update or []))
                out.append(inst)
            bb.instructions = out
    return n_new


def _build(L):
    """Build the Bass module for L GRU steps. Returns nc."""
    assert L >= 1
    nc = bass.Bass("TRN2", target_bir_lowering=False, debug=False)

    statet16_d = nc.dram_tensor("statet16", [126, 1024], F16, kind="ExternalInput").ap()
    statet8_d = nc.dram_tensor("statet8", [126, 1024], F8, kind="ExternalInput").ap()
    wn_d = nc.dram_tensor("wn", [2, 126, 4000], F16, kind="ExternalInput").ap()
    wrz_d = nc.dram_tensor("wrz", [2, 126, 4000], F8, kind="ExternalInput").ap()
    hout_d = nc.dram_tensor("hout", [128, HALF], F32, kind="ExternalOutput").ap()
    sums_d = nc.dram_tensor("sums", [128, L], F32, kind="ExternalOutput").ap()

    with tile.TileContext(nc) as tc:
        import contextlib
        with contextlib.ExitStack() as ctx:
            consts = ctx.enter_context(tc.tile_pool(name="consts", bufs=1))
            wpool = ctx.enter_context(tc.tile_pool(name="weights", bufs=1))
            hpool = ctx.enter_context(tc.tile_pool(name="hstate", bufs=1))
            work = ctx.enter_context(tc.tile_pool(name="work", bufs=2))
            gpsum = ctx.enter_context(tc.tile_pool(name="gpsum", bufs=2, space="PSUM"))
            tpsum = ctx.enter_context(tc.tile_pool(name="tpsum", bufs=1, space="PSUM"))

            identity = consts.tile([128, 128], F32, tag="identity", name="identity")
            make_identity(nc, identity[:])
            ident16 = consts.tile([128, 128], F16, tag="ident16", name="ident16")
            nc.vector.tensor_copy(ident16[:], identity[:])

            statet16 = wpool.tile([126, 1024], F16, tag="statet16", name="statet16")
            statet8 = wpool.tile([126, 1024], F8, tag="statet8", name="statet8")
            nc.gpsimd.dma_start(statet16[:], statet16_d)
            nc.gpsimd.dma_start(statet8[:], statet8_d)
            wn_t = [wpool.tile([126, 4000], F16, tag=f"wn{j}", name=f"wn{j}")
                    for j in range(2)]
            wrz_t = [wpool.tile([126, 4000], F8, tag=f"wrz{j}", name=f"wrz{j}")
                     for j in range(2)]
            nc.sync.dma_start(wrz_t[0][:], wrz_d[0])
            nc.gpsimd.dma_start(wn_t[0][:], wn_d[0])
            nc.sync.dma_start(wrz_t[1][:], wrz_d[1])
            nc.gpsimd.dma_start(wn_t[1][:], wn_d[1])

            hT16 = [hpool.tile([126, 1024], F16, tag=f"ht16{i}", name=f"ht16{i}")
                    for i in range(2)]
            hT8 = [hpool.tile([126, 1024], F8, tag=f"ht8{i}", name=f"ht8{i}")
                   for i in range(2)]
            h32 = [hpool.tile([128, HALF], F32, tag=f"h32{i}", name=f"h32{i}")
                   for i in range(2)]
            # zero the data rows; DMA row 125 (the bias ones live there) from
            # the state images (DVE ops can't start at partition 125, DMA can)
            for i in range(2):
                nc.vector.memzero(hT16[i][0:125, :])
                nc.vector.memzero(hT8[i][0:125, :])
                nc.gpsimd.dma_start(hT16[i][125:126, :], statet16_d[125:126, :])
                nc.gpsimd.dma_start(hT8[i][125:126, :], statet8_d[125:126, :])
            nc.vector.memzero(h32[1][:])

            sums = consts.tile([128, L], F32, tag="sums", name="sums")

            prev_zh16 = None
            prev_omzn16 = None
            for k in range(1, L + 1):
                first = k == 1
                rdbuf = k % 2
                dstbuf = (k + 1) % 2
                lhs16 = statet16 if first else hT16[rdbuf]
                lhs8 = statet8 if first else hT8[rdbuf]
                w16 = wn_t[0 if first else 1]
                w8 = wrz_t[0 if first else 1]

                rzp = gpsum.tile([128, 512], F32, tag="rzp", name="rzp")
                gnp = [gpsum.tile([128, 512], F32, tag=f"gnp{c}", name=f"gnp{c}")
                       for c in range(2)]

                def rz_mm(p, start, stop):
                    Kp = 126 if p < 2 else 125
                    lt = lhs8[0:Kp, 256 * p:256 * p + 256].rearrange(
                        "p (i c) -> p i c", c=128)
                    rt = w8[0:Kp, 1000 * p:1000 * p + 1000].rearrange(
                        "p (i n) -> p i n", n=500)
                    nc.tensor.matmul(rzp[:, 0:500], lt, rt, start=start,
                                     stop=stop, perf_mode=DR,
                                     skip_group_check=True)

                def n_mm(u, c, start, stop):
                    Ku = 126 if u in (1, 3) else 125
                    lt = lhs16[0:Ku, 128 * u:128 * u + 128]
                    rt = w16[0:Ku, 500 * u + 250 * c:500 * u + 250 * c + 250]
                    nc.tensor.matmul(gnp[c][:, 0:250], lt, rt, start=start,
                                     stop=stop, skip_group_check=True)

                def t_mm(dst, src_ap, start, stop):
                    nc.tensor.matmul(dst, src_ap, ident16[:], start=start,
                                     stop=stop, skip_group_check=True)

                def copies(tp, half, dst16, dst8):
                    """Copy the [125,128] transpose PSUM into the hT slot
                    halves: half 0 -> groups 0-3 / pairs 0-1 (cols 0:512),
                    half 1 -> groups 4-7 / pairs 2-3 (cols 512:1024). fp16
                    copies first: the N-group matmuls consume before RZ."""
                    tv = tp[:].rearrange("p (u c) -> p u c", c=64)
                    o = 512 * half
                    d0_16 = dst16[0:125, o:o + 256].rearrange(
                        "p (u c) -> p u c", c=128)[:, :, 0:64]
                    d1_16 = dst16[0:125, o + 256:o + 512].rearrange(
                        "p (u c) -> p u c", c=128)[:, :, 64:128]
                    d0_8 = dst8[0:125, o:o + 256].rearrange(
                        "p (i c) -> p i c", c=128)[:, :, 0:64]
                    d1_8 = dst8[0:125, o + 256:o + 512].rearrange(
                        "p (i c) -> p i c", c=128)[:, :, 64:128]
                    nc.scalar.copy(d0_16, tv)
                    nc.vector.tensor_copy(d1_16, tv)
                    nc.scalar.copy(d0_8, tv)
                    nc.vector.tensor_copy(d1_8, tv)

                # ---- PE stream: pA-consumers early, RZ mid (sigmoid
                # feeds the chain), chunk completions staggered; step k-1's
                # chunk-1 transposes+copies run inside this step's opening ----
                if prev_zh16 is not None:
                    tpB = tpsum.tile([125, 128], F32, tag="tpB", name="tpB")
                    t_mm(tpB[:], prev_zh16[:, 125:250], True, False)
                n_mm(0, 0, True, False)
                n_mm(1, 0, False, False)
                if prev_zh16 is not None:
                    t_mm(tpB[:], prev_omzn16[:, 125:250], False, True)
                    copies(tpB, 1, hT16[rdbuf], hT8[rdbuf])
                n_mm(2, 0, False, False)
                n_mm(3, 0, False, False)
                rz_mm(0, True, False)
                rz_mm(1, False, False)
                rz_mm(2, False, False)
                rz_mm(3, False, True)
                for u in (4, 5, 6, 7):
                    n_mm(u, 0, False, u == 7)      # <- chunk-0 complete
                for u in (0, 1, 2, 3):
                    n_mm(u, 1, u == 0, False)
                for u in (4, 5, 6, 7):
                    n_mm(u, 1, False, u == 7)      # <- chunk-1 complete

                # ---- gate chains: h_new = (1-z)*n + z*h, transposed form
                # T(h_new) = T(omz*n) + T(z*h); omz/zh off the critical path ----
                rz = work.tile([128, 2 * HALF], F32, tag="rz", name="rz")
                nc.scalar.activation(rz[:], rzp[:, 0:500], AF.Sigmoid)
                r = rz[:, 0:250]
                z = rz[:, 250:500]
                omz = work.tile([128, HALF], F32, tag="omz", name="omz")
                nc.gpsimd.tensor_scalar(omz[:], z, -1.0, 1.0,
                                        op0=ALU.mult, op1=ALU.add)
                zh16 = work.tile([128, HALF], F16, tag="zh16", name="zh16")
                nc.gpsimd.tensor_mul(zh16[:], z, h32[rdbuf][:])
                rhn = work.tile([128, HALF], F32, tag="rhn", name="rhn")
                targ = work.tile([128, HALF], F32, tag="targ", name="targ")
                n16 = work.tile([128, HALF], F16, tag="n16", name="n16")
                omzn16 = work.tile([128, HALF], F16, tag="omzn16", name="omzn16")
                for c in (0, 1):
                    cs = slice(125 * c, 125 * (c + 1))
                    nc.vector.tensor_mul(rhn[:, cs], r[:, cs], gnp[c][:, 125:250])
                    nc.vector.tensor_add(targ[:, cs], rhn[:, cs], gnp[c][:, 0:125])
                    nc.scalar.activation(n16[:, cs], targ[:, cs], AF.Tanh)
                    nc.vector.tensor_mul(omzn16[:, cs], omz[:, cs], n16[:, cs])
                # packed f32 h for the next step + break-check sums (off-path)
                nc.gpsimd.tensor_add(h32[dstbuf][:], omzn16[:], zh16[:])
                nc.vector.tensor_reduce(sums[:, k - 1:k], h32[dstbuf][:],
                                        axis=mybir.AxisListType.X, op=ALU.add)

                if k < L:
                    tpA = tpsum.tile([125, 128], F32, tag="tpA", name="tpA")
                    t_mm(tpA[:], zh16[:, 0:125], True, False)
                    t_mm(tpA[:], omzn16[:, 0:125], False, True)
                    copies(tpA, 0, hT16[dstbuf], hT8[dstbuf])
                    prev_zh16, prev_omzn16 = zh16, omzn16
                else:
                    prev_zh16 = prev_omzn16 = None

            nc.gpsimd.dma_start(hout_d, h32[(L + 1) % 2][:])
            nc.gpsimd.dma_start(sums_d, sums[:])

    _split_overwide_waits(nc)
    return nc


_NC_CACHE = {}


def _get_nc(L):
    if L not in _NC_CACHE:
        _NC_CACHE[L] = _build(L)
    return _NC_CACHE[L]


def _prep_weights(W_ih, W_hh, b_ih, b_hh):
    """Build the DRAM weight images: wrz [2, 126, 4000] fp8 (DoubleRow pair
    layout) and wn [2, 126, 4000] fp16 (chunk-interleaved n-gate layout).
    Index 0 = step-1 (x=state, h=0) weights, 1 = steady-state weights."""
    W_ih = np.asarray(W_ih, np.float32)
    W_hh = np.asarray(W_hh, np.float32)
    b_ih = np.asarray(b_ih, np.float32)
    b_hh = np.asarray(b_hh, np.float32)

    def rz_img(Wrz, brz):
        img = np.zeros((126, 4000), np.float32)
        for p in range(4):
            g = PAIR_G[p]
            rows = np.concatenate([np.arange(250 * g, 250 * g + 250),
                                   np.arange(500 + 250 * g, 500 + 250 * g + 250)])
            for i, t in enumerate(PAIR_T[p]):
                cols = PERM[125 * t:125 * (t + 1)]
                img[0:125, 1000 * p + 500 * i:1000 * p + 500 * i + 500] = \
                    Wrz[np.ix_(rows, cols)].T
            if p < 2:
                img[125, 1000 * p + 500:1000 * p + 1000] = brz[rows]
        return img.astype(F8NP)

    def n_img(Win, Whn, bin_, bhn):
        img = np.zeros((126, 4000), np.float32)
        for u in range(8):
            t, g = TBLK[u], GHALF[u]
            cols = PERM[125 * t:125 * (t + 1)]
            base = 500 * u
            for c in range(2):
                ch = np.arange(250 * g + 125 * c, 250 * g + 125 * c + 125)
                img[0:125, base + 250 * c:base + 250 * c + 125] = \
                    Win[np.ix_(ch, cols)].T
                img[0:125, base + 250 * c + 125:base + 250 * c + 250] = \
                    Whn[np.ix_(ch, cols)].T
                if u in (1, 3):
                    img[125, base + 250 * c:base + 250 * c + 125] = bin_[ch]
                    img[125, base + 250 * c + 125:base + 250 * c + 250] = bhn[ch]
        return img.astype(np.float16)

    Win = W_ih[1000:1500]
    Whn = W_hh[1000:1500]
    zeros_w = np.zeros_like(Whn)
    zeros_b = np.zeros(500, np.float32)
    wrz = np.stack([rz_img(W_ih[:1000], b_ih[:1000]),
                    rz_img(W_ih[:1000] + W_hh[:1000], b_ih[:1000] + b_hh[:1000])])
    wn = np.stack([n_img(Win, zeros_w, b_ih[1000:1500], zeros_b),
                   n_img(Win, Whn, b_ih[1000:1500], b_hh[1000:1500])])
    return np.ascontiguousarray(wrz), np.ascontiguousarray(wn)


def _prep_state(state):
    """Per-core stationary state^T images: fp16 [126, 1024] (group-major) and
    fp8 [126, 1024] (DoubleRow pair-major)."""
    state = np.asarray(state, np.float32)
    outs = []
    for cidx in range(NCORES):
        shard = state[BS * cidx:BS * (cidx + 1)]      # [64, 500]
        st = shard[:, PERM].T                          # [500, 64]
        s16 = np.zeros((126, 1024), np.float32)
        for u in range(8):
            t, g = TBLK[u], GHALF[u]
            s16[0:125, 128 * u + 64 * g:128 * u + 64 * g + 64] = \
                st[125 * t:125 * (t + 1)]
        s16[125, 128 * 1:128 * 1 + 64] = 1.0
        s16[125, 128 * 3 + 64:128 * 3 + 128] = 1.0
        s8 = np.zeros((126, 1024), np.float32)
        for p in range(4):
            g = PAIR_G[p]
            for i, t in enumerate(PAIR_T[p]):
                off = 256 * p + 128 * i + 64 * g
                s8[0:125, off:off + 64] = st[125 * t:125 * (t + 1)]
            if p < 2:
                s8[125, 256 * p + 128 + 64 * g:256 * p + 128 + 64 * g + 64] = 1.0
        outs.append((s16.astype(np.float16), s8.astype(F8NP)))
    return outs


def _run(L, stateTs, wrz, wn, trace=False):
    nc = _get_nc(L)
    in_maps = [{"statet16": np.ascontiguousarray(stateTs[c][0]),
                "statet8": np.ascontiguousarray(stateTs[c][1]),
                "wrz": wrz, "wn": wn} for c in range(NCORES)]
    res = bass_utils.run_bass_kernel_spmd(
        nc, in_maps, core_ids=list(range(NCORES)), trace=trace)
    shards = []
    sums = np.zeros((128, L), np.float64)
    for c in range(NCORES):
        hout = res.results[c]["hout"]
        shards.append(np.concatenate([hout[0:64], hout[64:128]], axis=1))
        sums += res.results[c]["sums"].astype(np.float64)
    h = np.concatenate(shards, axis=0)                # [512, 500]
    means = sums.sum(axis=0) / (B * D)                             # [L]
    return h, means, res


def kernel(state, W_ih, W_hh, b_ih, b_hh, break_condition, recursion_limit):
    state = np.asarray(state, np.float32)
    L = int(np.asarray(recursion_limit))
    if L <= 0:
        return state.copy()
    bc = float(np.asarray(break_condition))

    wrz, wn = _prep_weights(W_ih, W_hh, b_ih, b_hh)
    stateTs = _prep_state(state)

    h, means, _ = _run(L, stateTs, wrz, wn)
    fired = np.nonzero(means > bc)[0]
    if fired.size and fired[0] + 1 < L:
        # break fired at step k* = fired[0]+1: output latches h_{k*}
        h, _, _ = _run(int(fired[0]) + 1, stateTs, wrz, wn)
    return h.astype(np.float32)
